# revision 3
# baseline (speedup 1.0000x reference)
"""Bass/Trainium2 kernel for nn_DocRelPrompt.

Reference computation (B=64, L=512, H=768, HEAD=64, N_PROMPTS=10, N_LBL=2):
    rel2 = stack([1-r, r], 1)                   # (B, 2)
    hidden_rel = rel2 @ label_prompts           # (B, H)
    Q  = prompts @ ref_qw.T + ref_qb            # (10, HEAD)
    K  = hid @ ref_kw.T + ref_kb                # (B, L, HEAD)
    scores[b,n] = mean_l(Q[n] . K[b,l]) / 8
                = (hsum[b] . (Q@ref_kw)[n] / (512*8)) + (Q[n].ref_kb)/8
    gate = sigmoid(scores)                      # (B, 10)
    doc  = prompts[None] * gate[..., None]      # (B, 10, H)
    out  = concat([doc, hid + hidden_rel[:,None,:]], axis=1)   # (B, 522, H)

(The `_rel_prompts` branch of the reference is computed but unused, so it is
skipped entirely.)

Sharding: pure data-parallel over batch, 8 cores x 8 batches.  The tiny
prompt/weight tensors are folded on the host into W2s (768,10, bf16) and
c2 (10,) and replicated.

The kernel is memory-bound and the correctness gate is rel_err < 2e-2, so
the bulk streams run in bf16 end to end: the host downcasts hid to bf16
(halving the in-stream), the device computes body = hid + rel and the doc
gate entirely from the bf16 tiles, writes a bf16 out tensor (halving the
out-stream), and the host upcasts to f32.  Simulated numerics: rel err
~5.6e-3 vs the 2e-2 gate.

DRAM layout uses l = 4p + t ("(p t)") so every DMA touches per-partition
contiguous runs (6 KB full-tile / 3 KB half-tile) instead of 1.5 KB rows.

Device work per core, per batch (DMA-bound; ~13 MB HBM traffic total):
  - two half-tile hid loads (128, 2, 768) bf16 on the SP HWDGE ring, which
    carries nothing else (consts ride the ACT ring);
  - PE: hsum[b] (1, 768) via ones-stationary matmuls PSUM-accumulated over
    t-slices, ACT downcast, then 6 PE transposes build hsumT columns;
  - per-batch-PAIR gate tail: 6 bf16 matmuls accumulate scores, ACT
    sigmoid(+c2), DVE tensor_scalar doc rows (bf16), doc DMA;
  - DVE: rel row = db*r_b + lp0b (scalar_tensor_tensor, bf16), then per
    half-tile an in-place bf16 tensor_tensor body = hid + rel and a 0.4 MB
    body DMA on the ACT HWDGE ring.
"""

import numpy as np

B, L, H, HEAD, NPR, NLBL = 64, 512, 768, 64, 10, 2
NCORES = 8
BLOC = B // NCORES          # 8 batches per core
LT = L // 128               # 4 t-slices (l = 4p + t)
HC = H // 128               # 6 H-chunks of 128

_CACHE = {}


def _build_module():
    from contextlib import ExitStack

    import concourse.bacc as bacc
    import concourse.mybir as mybir
    from concourse.tile import TileContext

    dt = mybir.dt.float32
    bf = mybir.dt.bfloat16
    ADD = mybir.AluOpType.add

    # Bacc (not raw Bass): its compile() legalizes sync waits — TRN2
    # instructions carry at most one wait, extras become event-sem waits.
    nc = bacc.Bacc("TRN2", target_bir_lowering=False, debug=False)
    hid = nc.dram_tensor("hid", [BLOC, L, H], bf, kind="ExternalInput")
    # constants: constf (partition-broadcast rel ingredients, bf16),
    # prom (prompts, bf16), c2f (sigmoid bias, f32), w2st (folded score
    # weights, bf16) — all on the ACT ring so the SP ring is a pure hid
    # stream
    constf = nc.dram_tensor("constf", [128, 2 * H + BLOC], bf, kind="ExternalInput")
    prom = nc.dram_tensor("prom", [NPR, H], bf, kind="ExternalInput")
    c2f = nc.dram_tensor("c2f", [NPR, 1], dt, kind="ExternalInput")
    w2st = nc.dram_tensor("w2st", [128, HC * NPR], bf, kind="ExternalInput")
    out = nc.dram_tensor("out", [BLOC, NPR + L, H], bf, kind="ExternalOutput")

    # l = 4p + t: per-partition contiguous DRAM runs (4*H elems full tile)
    hid_r = hid[:].rearrange("b (p t) h -> b p t h", t=LT)
    body_r = out[:, NPR:, :].rearrange("b (p t) h -> b p t h", t=LT)

    with TileContext(nc) as tc, ExitStack() as ctx:
        const = ctx.enter_context(tc.tile_pool(name="const", bufs=1))
        hidp = ctx.enter_context(tc.tile_pool(name="hidp", bufs=8))
        relp = ctx.enter_context(tc.tile_pool(name="relp", bufs=3))
        hsp = ctx.enter_context(tc.tile_pool(name="hsp", bufs=2, space="PSUM"))
        hsbp = ctx.enter_context(tc.tile_pool(name="hsbp", bufs=2))
        sump = ctx.enter_context(tc.tile_pool(name="sump", bufs=1, space="PSUM"))
        scop = ctx.enter_context(tc.tile_pool(name="scop", bufs=2, space="PSUM"))
        warmp = ctx.enter_context(tc.tile_pool(name="warmp", bufs=1, space="PSUM"))
        small = ctx.enter_context(tc.tile_pool(name="small", bufs=1))

        ones_bf = const.tile([128, 1], bf)
        nc.vector.memset(ones_bf[:], 1.0)

        # the SP HWDGE ring is FIFO and carries only the hid stream; issue
        # the first loads immediately
        t_ins = []
        for b in range(2):
            t_in = hidp.tile([128, LT, H], bf, tag="hid")
            nc.sync.dma_start(t_in[:, 0:2], hid_r[b][:, 0:2])
            nc.sync.dma_start(t_in[:, 2:4], hid_r[b][:, 2:4])
            t_ins.append(t_in)

        # consts on the ACT ring (empty this early; first rel use is after
        # the first full hid load anyway)
        constf_sb = const.tile([128, 2 * H + BLOC], bf)
        nc.scalar.dma_start(constf_sb[:], constf[:])
        w2st_sb = const.tile([128, HC * NPR], bf)
        nc.scalar.dma_start(w2st_sb[:], w2st[:])
        prom_sb = const.tile([NPR, H], bf)
        nc.scalar.dma_start(prom_sb[:], prom[:])
        c2_sb = const.tile([NPR, 1], dt)
        nc.scalar.dma_start(c2_sb[:], c2f[:])
        lp0b_sb = constf_sb[:, 0:H]
        db_sb = constf_sb[:, H : 2 * H]
        rbc_sb = constf_sb[:, 2 * H : 2 * H + BLOC]

        # Warm-up matmuls: sync the PE against the DVE memset and the w2st
        # DMA one dependency at a time — matmuls tolerate few sync waits.
        scrap_ps = warmp.tile([128, 1], dt)
        nc.tensor.matmul(scrap_ps[0:1, :], ones_bf[:], ones_bf[:],
                         start=True, stop=True)
        nc.tensor.matmul(scrap_ps[0:NPR, :], w2st_sb[:, 0:NPR], ones_bf[:],
                         start=True, stop=True)
        # preload the sigmoid table during boot (1.3us ACT_TABLE_LOAD that
        # would otherwise land on the critical tail)
        sig_warm = small.tile([1, 1], dt)
        nc.scalar.activation(sig_warm[:], ones_bf[0:1, 0:1],
                             func=mybir.ActivationFunctionType.Sigmoid)

        # column c*BLOC+b = hsumT chunk; trailing pad dim keeps each bf16
        # transpose output column on a 4-byte PSUM boundary
        hsumT_ps = sump.tile([128, HC * BLOC, 2], bf)

        for b in range(BLOC):
            if b < 2:
                t_in = t_ins[b]
            else:
                t_in = hidp.tile([128, LT, H], bf, tag="hid")
                # half-tile loads: downstream compute starts earlier and
                # the outbound stream interleaves more smoothly
                nc.sync.dma_start(t_in[:, 0:2], hid_r[b][:, 0:2])
                nc.sync.dma_start(t_in[:, 2:4], hid_r[b][:, 2:4])

            # stage 1a: hsum (1, 768) = sum over (p, t) via ones-stationary
            # matmuls directly on the bf16 tile (PSUM accumulation over the
            # 4 t-slices; split 512/256 on the PSUM bank edge)
            hs_ps = hsp.tile([1, H], dt, tag="hs")
            for t in range(LT):
                nc.tensor.matmul(hs_ps[0:1, 0:512], ones_bf[:],
                                 t_in[:, t, 0:512],
                                 start=(t == 0), stop=(t == LT - 1))
                nc.tensor.matmul(hs_ps[0:1, 512:H], ones_bf[:],
                                 t_in[:, t, 512:H],
                                 start=(t == 0), stop=(t == LT - 1))

            # stage 1b: downcast hsum, transpose 128-chunks onto partitions.
            # Copies run on the DVE: the Scalar sequencer is the congested
            # queue (it dispatches the whole out-stream), and every ACT op
            # in the gate chain delays out-DMA dispatches behind it.
            hs_bf = hsbp.tile([1, H], bf, tag="hsbf")
            nc.vector.tensor_copy(hs_bf[:], hs_ps[:])
            for c in range(HC):
                col = c * BLOC + b
                nc.tensor.transpose(
                    hsumT_ps[:, col, 0:1],
                    hs_bf[0:1, c * 128 : (c + 1) * 128],
                    ones_bf[0:1, 0:1],
                )

            if b % 2 == 0:
                hsT_p = hsbp.tile([128, HC, 2], bf, tag="hstp")
            nc.vector.tensor_copy(hsT_p[:, :, b % 2], hsumT_ps[:, b :: BLOC, 0])

            # rel[b] = lp0 + r_b * (lp1 - lp0), already partition-broadcast on
            # the host; r_b enters as a per-partition scalar (DVE one op).
            rel_t = relp.tile([128, H], bf, tag="relsb")
            nc.vector.scalar_tensor_tensor(
                rel_t[:], db_sb, rbc_sb[:, b : b + 1], lp0b_sb,
                mybir.AluOpType.mult, ADD,
            )

            # body = hid + rel (in place, free-dim broadcast of rel over
            # t-slices), in halves so each outbound half-DMA starts as soon
            # as its add lands.  Mid-stream outs ride the ACT HWDGE ring so
            # they don't queue behind in-loads on the SP ring; the tail
            # batches split their halves across BOTH rings (the SP ring is
            # idle once the in-stream ends, and the two sequencers then
            # dispatch the trailing outs in parallel).
            for hlf in range(2):
                sl = slice(2 * hlf, 2 * hlf + 2)
                nc.vector.tensor_tensor(
                    t_in[:, sl], t_in[:, sl],
                    rel_t[:, None, :].broadcast_to([128, 2, H]),
                    ADD,
                )
                eng = nc.sync if (b >= 6 and hlf == 0) else nc.scalar
                eng.dma_start(body_r[b][:, sl], t_in[:, sl])

            # gate pipeline per batch PAIR — score columns are independent;
            # pairing halves the tiny stage-2 matmuls / sigmoids / doc DMAs.
            # Emitted AFTER the body block so a doc DMA never head-of-line
            # blocks the body outs behind it in the ring FIFO.
            if b % 2 == 1:
                score_p = scop.tile([NPR, 2], dt, tag="scorep")
                for c in range(HC):
                    nc.tensor.matmul(
                        score_p[:], w2st_sb[:, c * NPR : (c + 1) * NPR],
                        hsT_p[:, c, 0:2],
                        start=(c == 0), stop=(c == HC - 1),
                    )
                gate_p = hsbp.tile([NPR, 2], dt, tag="gatep")
                nc.scalar.activation(
                    gate_p[:], score_p[:],
                    func=mybir.ActivationFunctionType.Sigmoid,
                    bias=c2_sb, scale=1.0,
                )
                doc_p = hsbp.tile([NPR, 2, H], bf, tag="docp")
                for j in range(2):
                    nc.vector.tensor_scalar(
                        doc_p[:, j, :], prom_sb, gate_p[:, j : j + 1], None,
                        mybir.AluOpType.mult,
                    )
                eng = nc.sync if b == 7 else nc.scalar
                eng.dma_start(
                    out[b - 1 : b + 1, 0:NPR, :].transpose([1, 0, 2]), doc_p[:]
                )

    nc.compile()
    return nc


def _host_fold(prompts, label_prompts, qw, qb, kw, kb):
    """Fold the tiny projection weights on the host.

    scores[b, n] = hsum[b] . W2s[:, n] + c2[n], with W2s/c2 absorbing the
    1/L mean pooling and the 1/sqrt(HEAD) scaling.
    """
    q = prompts.astype(np.float64) @ qw.astype(np.float64).T + qb.astype(np.float64)
    w2 = q @ kw.astype(np.float64)                               # (10, H)
    w2s = (w2.T / (L * np.sqrt(HEAD))).astype(np.float32)        # (H, 10)
    c2 = ((q @ kb.astype(np.float64)) / np.sqrt(HEAD)).astype(np.float32)  # (10,)
    # device layout: (128, HC*NPR), free index = c*NPR + n for h = c*128 + p
    import ml_dtypes

    w2st = np.ascontiguousarray(
        w2s.reshape(HC, 128, NPR).transpose(1, 0, 2).reshape(128, HC * NPR)
    ).astype(ml_dtypes.bfloat16)
    return w2st, c2.reshape(NPR, 1)


def _prepare_in_maps(
    relevance, hidden_states_src, prompts, label_prompts,
    ref_qw, ref_qb, ref_kw, ref_kb, **_unused,
):
    import ml_dtypes

    bf16 = ml_dtypes.bfloat16
    relevance = np.asarray(relevance, dtype=np.float32)
    hidden_states_src = np.asarray(hidden_states_src, dtype=np.float32)
    prompts = np.asarray(prompts, dtype=np.float32)
    label_prompts = np.asarray(label_prompts, dtype=np.float32)

    w2st, c2 = _host_fold(
        prompts, label_prompts,
        np.asarray(ref_qw, np.float32), np.asarray(ref_qb, np.float32),
        np.asarray(ref_kw, np.float32), np.asarray(ref_kb, np.float32),
    )
    dvec = label_prompts[1] - label_prompts[0]
    prom_bf = np.ascontiguousarray(prompts).astype(bf16)
    hid_bf = hidden_states_src.astype(bf16)

    in_maps = []
    for core in range(NCORES):
        sl = slice(core * BLOC, (core + 1) * BLOC)
        constf = np.empty((128, 2 * H + BLOC), np.float32)
        constf[:, 0:H] = label_prompts[0]
        constf[:, H : 2 * H] = dvec
        constf[:, 2 * H :] = relevance[sl]
        in_maps.append(
            {
                "hid": np.ascontiguousarray(hid_bf[sl]),
                "constf": constf.astype(bf16),
                "prom": prom_bf,
                "c2f": c2,
                "w2st": w2st,
            }
        )
    return in_maps


def _get_module():
    if "nc" not in _CACHE:
        _CACHE["nc"] = _build_module()
    return _CACHE["nc"]


def kernel(**inputs):
    from concourse.bass_utils import run_bass_kernel_spmd

    nc = _get_module()
    in_maps = _prepare_in_maps(**inputs)
    res = run_bass_kernel_spmd(nc, in_maps, list(range(NCORES)))
    return np.concatenate(
        [res.results[c]["out"] for c in range(NCORES)], axis=0
    ).astype(np.float32)


# revision 8
# speedup vs baseline: 1.0154x; 1.0154x over previous
"""Bass/Trainium2 kernel for nn_DocRelPrompt.

Reference computation (B=64, L=512, H=768, HEAD=64, N_PROMPTS=10, N_LBL=2):
    rel2 = stack([1-r, r], 1)                   # (B, 2)
    hidden_rel = rel2 @ label_prompts           # (B, H)
    Q  = prompts @ ref_qw.T + ref_qb            # (10, HEAD)
    K  = hid @ ref_kw.T + ref_kb                # (B, L, HEAD)
    scores[b,n] = mean_l(Q[n] . K[b,l]) / 8
                = (hsum[b] . (Q@ref_kw)[n] / (512*8)) + (Q[n].ref_kb)/8
    gate = sigmoid(scores)                      # (B, 10)
    doc  = prompts[None] * gate[..., None]      # (B, 10, H)
    out  = concat([doc, hid + hidden_rel[:,None,:]], axis=1)   # (B, 522, H)

(The `_rel_prompts` branch of the reference is computed but unused, so it is
skipped entirely.)

Sharding: pure data-parallel over batch, 8 cores x 8 batches.  The tiny
prompt/weight tensors are folded on the host into W2s (768,10, bf16) and
c2 (10,) and replicated.

The kernel is memory-bound and the correctness gate is rel_err < 2e-2, so
the bulk streams run in bf16 end to end: the host downcasts hid to bf16
(halving the in-stream), the device computes body = hid + rel and the doc
gate entirely from the bf16 tiles, writes a bf16 out tensor (halving the
out-stream), and the host upcasts to f32.  Simulated numerics: rel err
~5.6e-3 vs the 2e-2 gate.

DRAM layout uses l = 4p + t ("(p t)") so every DMA touches per-partition
contiguous runs (6 KB full-tile / 3 KB half-tile) instead of 1.5 KB rows.

Device work per core, per batch (DMA-bound; ~13 MB HBM traffic total):
  - two half-tile hid loads (128, 2, 768) bf16 on the SP HWDGE ring, which
    carries nothing else (consts ride the ACT ring);
  - PE: hsum[b] (1, 768) via ones-stationary matmuls PSUM-accumulated over
    t-slices, ACT downcast, then 6 PE transposes build hsumT columns;
  - per-batch-PAIR gate tail: 6 bf16 matmuls accumulate scores, ACT
    sigmoid(+c2), DVE tensor_scalar doc rows (bf16), doc DMA;
  - DVE: rel row = db*r_b + lp0b (scalar_tensor_tensor, bf16), then per
    half-tile an in-place bf16 tensor_tensor body = hid + rel and a 0.4 MB
    body DMA on the ACT HWDGE ring.
"""

import numpy as np

B, L, H, HEAD, NPR, NLBL = 64, 512, 768, 64, 10, 2
NCORES = 8
BLOC = B // NCORES          # 8 batches per core
LT = L // 128               # 4 t-slices (l = 4p + t)
HC = H // 128               # 6 H-chunks of 128

_CACHE = {}


def _build_module():
    from contextlib import ExitStack

    import concourse.bacc as bacc
    import concourse.mybir as mybir
    from concourse.tile import TileContext

    dt = mybir.dt.float32
    bf = mybir.dt.bfloat16
    ADD = mybir.AluOpType.add

    # Bacc (not raw Bass): its compile() legalizes sync waits — TRN2
    # instructions carry at most one wait, extras become event-sem waits.
    nc = bacc.Bacc("TRN2", target_bir_lowering=False, debug=False)
    hid = nc.dram_tensor("hid", [BLOC, L, H], bf, kind="ExternalInput")
    # constants: constf (partition-broadcast rel ingredients, bf16),
    # prom (prompts, bf16), c2f (sigmoid bias, f32), w2st (folded score
    # weights, bf16) — all on the ACT ring so the SP ring is a pure hid
    # stream
    constf = nc.dram_tensor("constf", [128, 2 * H + BLOC], bf, kind="ExternalInput")
    prom = nc.dram_tensor("prom", [NPR, H], bf, kind="ExternalInput")
    c2f = nc.dram_tensor("c2f", [NPR, 1], dt, kind="ExternalInput")
    w2st = nc.dram_tensor("w2st", [128, HC * NPR], bf, kind="ExternalInput")
    out = nc.dram_tensor("out", [BLOC, NPR + L, H], bf, kind="ExternalOutput")

    # l = 4p + t: per-partition contiguous DRAM runs (4*H elems full tile)
    hid_r = hid[:].rearrange("b (p t) h -> b p t h", t=LT)
    body_r = out[:, NPR:, :].rearrange("b (p t) h -> b p t h", t=LT)

    with TileContext(nc) as tc, ExitStack() as ctx:
        const = ctx.enter_context(tc.tile_pool(name="const", bufs=1))
        hidp = ctx.enter_context(tc.tile_pool(name="hidp", bufs=8))
        bodyp = ctx.enter_context(tc.tile_pool(name="bodyp", bufs=16))
        relp = ctx.enter_context(tc.tile_pool(name="relp", bufs=3))
        hsp = ctx.enter_context(tc.tile_pool(name="hsp", bufs=2, space="PSUM"))
        hsbp = ctx.enter_context(tc.tile_pool(name="hsbp", bufs=2))
        sump = ctx.enter_context(tc.tile_pool(name="sump", bufs=1, space="PSUM"))
        scop = ctx.enter_context(tc.tile_pool(name="scop", bufs=2, space="PSUM"))
        warmp = ctx.enter_context(tc.tile_pool(name="warmp", bufs=1, space="PSUM"))
        small = ctx.enter_context(tc.tile_pool(name="small", bufs=1))

        ones_bf = const.tile([128, 1], bf)
        nc.vector.memset(ones_bf[:], 1.0)

        # the SP HWDGE ring is FIFO and carries only the hid stream; issue
        # the first loads immediately
        t_ins = []
        for b in range(2):
            t_in = hidp.tile([128, LT, H], bf, tag="hid")
            nc.sync.dma_start(t_in[:, 0:2], hid_r[b][:, 0:2])
            nc.sync.dma_start(t_in[:, 2:4], hid_r[b][:, 2:4])
            t_ins.append(t_in)

        # consts on the ACT ring (empty this early; first rel use is after
        # the first full hid load anyway)
        constf_sb = const.tile([128, 2 * H + BLOC], bf)
        nc.scalar.dma_start(constf_sb[:], constf[:])
        w2st_sb = const.tile([128, HC * NPR], bf)
        nc.scalar.dma_start(w2st_sb[:], w2st[:])
        prom_sb = const.tile([NPR, H], bf)
        nc.scalar.dma_start(prom_sb[:], prom[:])
        c2_sb = const.tile([NPR, 1], dt)
        nc.scalar.dma_start(c2_sb[:], c2f[:])
        lp0b_sb = constf_sb[:, 0:H]
        db_sb = constf_sb[:, H : 2 * H]
        rbc_sb = constf_sb[:, 2 * H : 2 * H + BLOC]

        # Warm-up matmuls: sync the PE against the DVE memset and the w2st
        # DMA one dependency at a time — matmuls tolerate few sync waits.
        scrap_ps = warmp.tile([128, 1], dt)
        nc.tensor.matmul(scrap_ps[0:1, :], ones_bf[:], ones_bf[:],
                         start=True, stop=True)
        nc.tensor.matmul(scrap_ps[0:NPR, :], w2st_sb[:, 0:NPR], ones_bf[:],
                         start=True, stop=True)
        # preload the sigmoid table during boot (1.3us ACT_TABLE_LOAD that
        # would otherwise land on the critical tail)
        sig_warm = small.tile([1, 1], dt)
        nc.scalar.activation(sig_warm[:], ones_bf[0:1, 0:1],
                             func=mybir.ActivationFunctionType.Sigmoid)

        # column c*BLOC+b = hsumT chunk; trailing pad dim keeps each bf16
        # transpose output column on a 4-byte PSUM boundary
        hsumT_ps = sump.tile([128, HC * BLOC, 2], bf)

        for b in range(BLOC):
            if b < 2:
                t_in = t_ins[b]
            else:
                t_in = hidp.tile([128, LT, H], bf, tag="hid")
                # half-tile loads: downstream compute starts earlier and
                # the outbound stream interleaves more smoothly
                nc.sync.dma_start(t_in[:, 0:2], hid_r[b][:, 0:2])
                nc.sync.dma_start(t_in[:, 2:4], hid_r[b][:, 2:4])

            # stage 1a: hsum (1, 768) = sum over (p, t) via ones-stationary
            # matmuls directly on the bf16 tile (PSUM accumulation over the
            # 4 t-slices; split 512/256 on the PSUM bank edge)
            hs_ps = hsp.tile([1, H], dt, tag="hs")
            for t in range(LT):
                nc.tensor.matmul(hs_ps[0:1, 0:512], ones_bf[:],
                                 t_in[:, t, 0:512],
                                 start=(t == 0), stop=(t == LT - 1))
                nc.tensor.matmul(hs_ps[0:1, 512:H], ones_bf[:],
                                 t_in[:, t, 512:H],
                                 start=(t == 0), stop=(t == LT - 1))

            # rel[b] = lp0 + r_b * (lp1 - lp0), already partition-broadcast on
            # the host; r_b enters as a per-partition scalar (DVE one op).
            rel_t = relp.tile([128, H], bf, tag="relsb")
            nc.vector.scalar_tensor_tensor(
                rel_t[:], db_sb, rbc_sb[:, b : b + 1], lp0b_sb,
                mybir.AluOpType.mult, ADD,
            )

            # body = hid + rel (separate output tile, NOT in place: an
            # in-place add has a WAR hazard against the PE hsum reads of the
            # same buffer, which would chain PE latency into the body path),
            # free-dim broadcast of rel over t-slices.  Mid-stream outs ride
            # the ACT HWDGE ring so they don't queue behind in-loads on the
            # SP ring; the tail splits across BOTH rings (the SP ring is
            # idle once the in-stream ends) and the last half goes in
            # quarters so the final transfer chases a half-size add.
            chunks = [(slice(0, 2), nc.scalar), (slice(2, 4), nc.scalar)]
            if b == 7:
                chunks = [(slice(0, 2), nc.scalar), (slice(2, 3), nc.scalar),
                          (slice(3, 4), nc.sync)]
            elif b == 6:
                chunks = [(slice(0, 2), nc.sync), (slice(2, 4), nc.scalar)]
            for sl, eng in chunks:
                n = sl.stop - sl.start
                t_body = bodyp.tile([128, n, H], bf, tag=f"body{n}")
                nc.vector.tensor_tensor(
                    t_body[:], t_in[:, sl],
                    rel_t[:, None, :].broadcast_to([128, n, H]),
                    ADD,
                )
                eng.dma_start(body_r[b][:, sl], t_body[:])

            # stage 1b: downcast hsum, transpose 128-chunks onto partitions.
            # The copies stay on ACT (the DVE queue carries the
            # latency-critical adds) and are emitted AFTER the body block so
            # they never head-of-line block an out-DMA dispatch behind them
            # in the Scalar queue — sequencers execute strictly in order,
            # and hs_bf waits on the full-tile PE hsum.
            hs_bf = hsbp.tile([1, H], bf, tag="hsbf")
            nc.scalar.copy(hs_bf[:], hs_ps[:])
            for c in range(HC):
                col = c * BLOC + b
                nc.tensor.transpose(
                    hsumT_ps[:, col, 0:1],
                    hs_bf[0:1, c * 128 : (c + 1) * 128],
                    ones_bf[0:1, 0:1],
                )

            if b % 2 == 0:
                hsT_p = hsbp.tile([128, HC, 2], bf, tag="hstp")
            nc.scalar.copy(hsT_p[:, :, b % 2], hsumT_ps[:, b :: BLOC, 0])

            # gate pipeline per batch PAIR — score columns are independent;
            # pairing halves the tiny stage-2 matmuls / sigmoids / doc DMAs.
            # Emitted AFTER the body block so a doc DMA never head-of-line
            # blocks the body outs behind it in the ring FIFO.
            if b % 2 == 1:
                score_p = scop.tile([NPR, 2], dt, tag="scorep")
                for c in range(HC):
                    nc.tensor.matmul(
                        score_p[:], w2st_sb[:, c * NPR : (c + 1) * NPR],
                        hsT_p[:, c, 0:2],
                        start=(c == 0), stop=(c == HC - 1),
                    )
                gate_p = hsbp.tile([NPR, 2], dt, tag="gatep")
                nc.scalar.activation(
                    gate_p[:], score_p[:],
                    func=mybir.ActivationFunctionType.Sigmoid,
                    bias=c2_sb, scale=1.0,
                )
                doc_p = hsbp.tile([NPR, 2, H], bf, tag="docp")
                for j in range(2):
                    nc.vector.tensor_scalar(
                        doc_p[:, j, :], prom_sb, gate_p[:, j : j + 1], None,
                        mybir.AluOpType.mult,
                    )
                eng = nc.sync if b == 7 else nc.scalar
                eng.dma_start(
                    out[b - 1 : b + 1, 0:NPR, :].transpose([1, 0, 2]), doc_p[:]
                )

    nc.compile()
    return nc


def _host_fold(prompts, label_prompts, qw, qb, kw, kb):
    """Fold the tiny projection weights on the host.

    scores[b, n] = hsum[b] . W2s[:, n] + c2[n], with W2s/c2 absorbing the
    1/L mean pooling and the 1/sqrt(HEAD) scaling.
    """
    q = prompts.astype(np.float64) @ qw.astype(np.float64).T + qb.astype(np.float64)
    w2 = q @ kw.astype(np.float64)                               # (10, H)
    w2s = (w2.T / (L * np.sqrt(HEAD))).astype(np.float32)        # (H, 10)
    c2 = ((q @ kb.astype(np.float64)) / np.sqrt(HEAD)).astype(np.float32)  # (10,)
    # device layout: (128, HC*NPR), free index = c*NPR + n for h = c*128 + p
    import ml_dtypes

    w2st = np.ascontiguousarray(
        w2s.reshape(HC, 128, NPR).transpose(1, 0, 2).reshape(128, HC * NPR)
    ).astype(ml_dtypes.bfloat16)
    return w2st, c2.reshape(NPR, 1)


def _prepare_in_maps(
    relevance, hidden_states_src, prompts, label_prompts,
    ref_qw, ref_qb, ref_kw, ref_kb, **_unused,
):
    import ml_dtypes

    bf16 = ml_dtypes.bfloat16
    relevance = np.asarray(relevance, dtype=np.float32)
    hidden_states_src = np.asarray(hidden_states_src, dtype=np.float32)
    prompts = np.asarray(prompts, dtype=np.float32)
    label_prompts = np.asarray(label_prompts, dtype=np.float32)

    w2st, c2 = _host_fold(
        prompts, label_prompts,
        np.asarray(ref_qw, np.float32), np.asarray(ref_qb, np.float32),
        np.asarray(ref_kw, np.float32), np.asarray(ref_kb, np.float32),
    )
    dvec = label_prompts[1] - label_prompts[0]
    prom_bf = np.ascontiguousarray(prompts).astype(bf16)
    hid_bf = hidden_states_src.astype(bf16)

    in_maps = []
    for core in range(NCORES):
        sl = slice(core * BLOC, (core + 1) * BLOC)
        constf = np.empty((128, 2 * H + BLOC), np.float32)
        constf[:, 0:H] = label_prompts[0]
        constf[:, H : 2 * H] = dvec
        constf[:, 2 * H :] = relevance[sl]
        in_maps.append(
            {
                "hid": np.ascontiguousarray(hid_bf[sl]),
                "constf": constf.astype(bf16),
                "prom": prom_bf,
                "c2f": c2,
                "w2st": w2st,
            }
        )
    return in_maps


def _get_module():
    if "nc" not in _CACHE:
        _CACHE["nc"] = _build_module()
    return _CACHE["nc"]


def kernel(**inputs):
    from concourse.bass_utils import run_bass_kernel_spmd

    nc = _get_module()
    in_maps = _prepare_in_maps(**inputs)
    res = run_bass_kernel_spmd(nc, in_maps, list(range(NCORES)))
    return np.concatenate(
        [res.results[c]["out"] for c in range(NCORES)], axis=0
    ).astype(np.float32)


# revision 17
# speedup vs baseline: 1.0663x; 1.0502x over previous
"""Bass/Trainium2 kernel for nn_DocRelPrompt.

Reference computation (B=64, L=512, H=768, HEAD=64, N_PROMPTS=10, N_LBL=2):
    rel2 = stack([1-r, r], 1)                   # (B, 2)
    hidden_rel = rel2 @ label_prompts           # (B, H)
    Q  = prompts @ ref_qw.T + ref_qb            # (10, HEAD)
    K  = hid @ ref_kw.T + ref_kb                # (B, L, HEAD)
    scores[b,n] = mean_l(Q[n] . K[b,l]) / 8
                = (hsum[b] . (Q@ref_kw)[n] / (512*8)) + (Q[n].ref_kb)/8
    gate = sigmoid(scores)                      # (B, 10)
    doc  = prompts[None] * gate[..., None]      # (B, 10, H)
    out  = concat([doc, hid + hidden_rel[:,None,:]], axis=1)   # (B, 522, H)

(The `_rel_prompts` branch of the reference is computed but unused, so it is
skipped entirely.)

Sharding: pure data-parallel over batch, 8 cores x 8 batches.  The tiny
prompt/weight tensors are folded on the host into W2s (768,10, bf16) and
c2 (10,) and replicated.

The kernel is memory-bound and the correctness gate is rel_err < 2e-2, so
the bulk streams run in bf16 end to end: the host downcasts hid to bf16
(halving the in-stream), the device computes body = hid + rel and the doc
gate entirely from the bf16 tiles, writes a bf16 out tensor (halving the
out-stream), and the host upcasts to f32.  Simulated numerics: rel err
~5.6e-3 vs the 2e-2 gate.

DRAM layout uses l = 4p + t ("(p t)") so every DMA touches per-partition
contiguous runs (6 KB full-tile / 3 KB half-tile) instead of 1.5 KB rows.

Device work per core, per batch (DMA-bound; ~13 MB HBM traffic total):
  - two half-tile hid loads (128, 2, 768) bf16 on the SP HWDGE ring, which
    carries nothing else (consts ride the ACT ring);
  - PE: hsum[b] (1, 768) via ones-stationary matmuls PSUM-accumulated over
    t-slices, ACT downcast, then 6 PE transposes build hsumT columns;
  - per-batch-PAIR gate tail: 6 bf16 matmuls accumulate scores, ACT
    sigmoid(+c2), DVE tensor_scalar doc rows (bf16), doc DMA;
  - DVE: rel row = db*r_b + lp0b (scalar_tensor_tensor, bf16), then per
    half-tile an in-place bf16 tensor_tensor body = hid + rel and a 0.4 MB
    body DMA on the ACT HWDGE ring.
"""

import numpy as np

B, L, H, HEAD, NPR, NLBL = 64, 512, 768, 64, 10, 2
NCORES = 8
BLOC = B // NCORES          # 8 batches per core
LT = L // 128               # 4 t-slices (l = 4p + t)
HC = H // 128               # 6 H-chunks of 128

_CACHE = {}


def _build_module():
    from contextlib import ExitStack

    import concourse.bacc as bacc
    import concourse.mybir as mybir
    from concourse.tile import TileContext

    dt = mybir.dt.float32
    bf = mybir.dt.bfloat16
    ADD = mybir.AluOpType.add

    # Bacc (not raw Bass): its compile() legalizes sync waits — TRN2
    # instructions carry at most one wait, extras become event-sem waits.
    nc = bacc.Bacc("TRN2", target_bir_lowering=False, debug=False)
    hid = nc.dram_tensor("hid", [BLOC, L, H], bf, kind="ExternalInput")
    # constants: relr (the 8 per-batch rel rows, host-computed, 12 KB —
    # partition-broadcast on-chip per batch via PE ones-matmuls instead of
    # streaming a 395 KB pre-broadcast tensor from HBM), prom (prompts,
    # bf16), c2f (sigmoid bias, f32), w2st (folded score weights, bf16) —
    # all on the ACT ring so the SP ring is a pure hid stream
    relr = nc.dram_tensor("relr", [1, BLOC * H], bf, kind="ExternalInput")
    prom = nc.dram_tensor("prom", [NPR, H], bf, kind="ExternalInput")
    c2f = nc.dram_tensor("c2f", [NPR, 1], dt, kind="ExternalInput")
    w2st = nc.dram_tensor("w2st", [128, HC * NPR], bf, kind="ExternalInput")
    out = nc.dram_tensor("out", [BLOC, NPR + L, H], bf, kind="ExternalOutput")

    # l = 4p + t: per-partition contiguous DRAM runs (4*H elems full tile)
    hid_r = hid[:].rearrange("b (p t) h -> b p t h", t=LT)
    body_r = out[:, NPR:, :].rearrange("b (p t) h -> b p t h", t=LT)

    with TileContext(nc) as tc, ExitStack() as ctx:
        const = ctx.enter_context(tc.tile_pool(name="const", bufs=1))
        hidp = ctx.enter_context(tc.tile_pool(name="hidp", bufs=8))
        bodyp = ctx.enter_context(tc.tile_pool(name="bodyp", bufs=14))
        relp = ctx.enter_context(tc.tile_pool(name="relp", bufs=3))
        hstp = ctx.enter_context(tc.tile_pool(name="hstp", bufs=2, space="PSUM"))
        hsbp = ctx.enter_context(tc.tile_pool(name="hsbp", bufs=2))
        rbcp = ctx.enter_context(tc.tile_pool(name="rbcp", bufs=1, space="PSUM"))
        scop = ctx.enter_context(tc.tile_pool(name="scop", bufs=2, space="PSUM"))
        warmp = ctx.enter_context(tc.tile_pool(name="warmp", bufs=1, space="PSUM"))
        small = ctx.enter_context(tc.tile_pool(name="small", bufs=1))

        ones_bf = const.tile([128, 1], bf)
        nc.vector.memset(ones_bf[:], 1.0)
        ones_row = const.tile([1, 128], bf)
        nc.vector.memset(ones_row[:], 1.0)

        # the SP HWDGE ring is FIFO and carries only the hid stream; issue
        # the first loads immediately
        t_ins = []
        for b in range(2):
            t_in = hidp.tile([128, LT, H], bf, tag="hid")
            nc.sync.dma_start(t_in[:, 0:2], hid_r[b][:, 0:2])
            nc.sync.dma_start(t_in[:, 2:4], hid_r[b][:, 2:4])
            t_ins.append(t_in)

        # consts on the ACT ring (empty this early; first rel use is after
        # the first full hid load anyway)
        relr_sb = const.tile([1, BLOC * H], bf)
        nc.scalar.dma_start(relr_sb[:], relr[:])
        w2st_sb = const.tile([128, HC * NPR], bf)
        nc.scalar.dma_start(w2st_sb[:], w2st[:])
        prom_sb = const.tile([NPR, H], bf)
        nc.scalar.dma_start(prom_sb[:], prom[:])
        c2_sb = const.tile([NPR, 1], dt)
        nc.scalar.dma_start(c2_sb[:], c2f[:])

        # Warm-up matmuls: sync the PE against the DVE memsets and the w2st
        # DMA one dependency at a time — matmuls tolerate few sync waits.
        scrap_ps = warmp.tile([128, 1], dt)
        nc.tensor.matmul(scrap_ps[0:1, :], ones_bf[:], ones_bf[:],
                         start=True, stop=True)
        nc.tensor.matmul(scrap_ps[0:NPR, :], w2st_sb[:, 0:NPR], ones_bf[:],
                         start=True, stop=True)
        # preload the sigmoid table during boot (1.3us ACT_TABLE_LOAD that
        # would otherwise land on the critical tail)
        sig_warm = small.tile([1, 1], dt)
        nc.scalar.activation(sig_warm[:], ones_bf[0:1, 0:1],
                             func=mybir.ActivationFunctionType.Sigmoid)

        # all doc rows accumulate here; ONE doc DMA at the end of the kernel
        # (4 interleaved doc DMAs would each cost a serialized ~0.6us
        # dispatch slot in the congested out-queue)
        doc_all = const.tile([NPR, BLOC, H], bf)

        for b in range(BLOC):
            if b < 2:
                t_in = t_ins[b]
            else:
                t_in = hidp.tile([128, LT, H], bf, tag="hid")
                # half-tile loads: downstream compute starts earlier and
                # the outbound stream interleaves more smoothly
                nc.sync.dma_start(t_in[:, 0:2], hid_r[b][:, 0:2])
                nc.sync.dma_start(t_in[:, 2:4], hid_r[b][:, 2:4])

            # rel[b] (host-computed row) partition-broadcast via PE
            # ones-matmuls (f32 PSUM, split 512/256 on the bank edge), then
            # one ACT downcast to a bf16 SBUF tile for the DVE adds.
            # Emitted BEFORE the hsum chunks: it depends only on boot-time
            # consts, and the in-order PE queue must produce it before the
            # first body add needs it.
            rel_ps = rbcp.tile([128, H], dt, tag="relps")
            nc.tensor.matmul(rel_ps[:, 0:512], ones_row[:],
                             relr_sb[0:1, b * H : b * H + 512],
                             start=True, stop=True)
            nc.tensor.matmul(rel_ps[:, 512:H], ones_row[:],
                             relr_sb[0:1, b * H + 512 : (b + 1) * H],
                             start=True, stop=True)
            rel_t = relp.tile([128, H], bf, tag="relsb")
            nc.scalar.copy(rel_t[:], rel_ps[:])

            # stage 1a: hsumT (128, HC) — hid-chunk-stationary matmuls with
            # a ones moving vector put the h-chunk sums directly ON the
            # partitions (PSUM f32 accumulation over the 4 t-slices).  This
            # skips the (1,768) ACT downcast + 6 PE transposes of the
            # classic hsum formulation — the ACT queue is the congested one
            # and those copies sat in the last batch's critical chain.
            hsT_ps = hstp.tile([128, HC], dt, tag="hsT")
            for c in range(HC):
                for t in range(LT):
                    nc.tensor.matmul(
                        hsT_ps[:, c : c + 1],
                        t_in[:, t, c * 128 : (c + 1) * 128],
                        ones_bf[:],
                        start=(t == 0), stop=(t == LT - 1),
                    )

            # body = hid + rel (separate output tile, NOT in place: an
            # in-place add has a WAR hazard against the PE hsum reads of the
            # same buffer, which would chain PE latency into the body path),
            # free-dim broadcast of rel over t-slices.  Mid-stream outs ride
            # the ACT HWDGE ring so they don't queue behind in-loads on the
            # SP ring; the tail splits across BOTH rings (the SP ring is
            # idle once the in-stream ends) and the last half goes in
            # quarters so the final transfer chases a half-size add.
            chunks = [(slice(0, 2), nc.scalar), (slice(2, 4), nc.scalar)]
            if b == 7:
                chunks = [(slice(0, 2), nc.scalar), (slice(2, 3), nc.scalar),
                          (slice(3, 4), nc.sync)]
            elif b == 6:
                chunks = [(slice(0, 2), nc.sync), (slice(2, 4), nc.scalar)]
            for sl, eng in chunks:
                n = sl.stop - sl.start
                t_body = bodyp.tile([128, n, H], bf, tag=f"body{n}")
                nc.vector.tensor_tensor(
                    t_body[:], t_in[:, sl],
                    rel_t[:, None, :].broadcast_to([128, n, H]),
                    ADD,
                )
                eng.dma_start(body_r[b][:, sl], t_body[:])

            # stage 1b: one tiny ACT downcast (6 elems/partition) per batch,
            # emitted AFTER the body block so it never head-of-line blocks
            # an out-DMA dispatch behind it in the Scalar queue.
            if b % 2 == 0:
                hsT_p = hsbp.tile([128, HC, 2], bf, tag="hstp")
            nc.scalar.copy(hsT_p[:, :, b % 2], hsT_ps[:])

            # gate pipeline per batch PAIR — score columns are independent;
            # pairing halves the tiny stage-2 matmuls / sigmoids.
            if b % 2 == 1:
                score_p = scop.tile([NPR, 2], dt, tag="scorep")
                for c in range(HC):
                    nc.tensor.matmul(
                        score_p[:], w2st_sb[:, c * NPR : (c + 1) * NPR],
                        hsT_p[:, c, 0:2],
                        start=(c == 0), stop=(c == HC - 1),
                    )
                gate_p = hsbp.tile([NPR, 2], dt, tag="gatep")
                nc.scalar.activation(
                    gate_p[:], score_p[:],
                    func=mybir.ActivationFunctionType.Sigmoid,
                    bias=c2_sb, scale=1.0,
                )
                for j in range(2):
                    nc.vector.tensor_scalar(
                        doc_all[:, b - 1 + j, :], prom_sb,
                        gate_p[:, j : j + 1], None,
                        mybir.AluOpType.mult,
                    )

        # single doc DMA for all 8 batches, on the SP ring (idle by now —
        # the in-stream is done well before the last gate chain resolves)
        nc.sync.dma_start(
            out[:, 0:NPR, :].transpose([1, 0, 2]), doc_all[:]
        )

    nc.compile()
    return nc


def _host_fold(prompts, label_prompts, qw, qb, kw, kb):
    """Fold the tiny projection weights on the host.

    scores[b, n] = hsum[b] . W2s[:, n] + c2[n], with W2s/c2 absorbing the
    1/L mean pooling and the 1/sqrt(HEAD) scaling.
    """
    q = prompts.astype(np.float64) @ qw.astype(np.float64).T + qb.astype(np.float64)
    w2 = q @ kw.astype(np.float64)                               # (10, H)
    w2s = (w2.T / (L * np.sqrt(HEAD))).astype(np.float32)        # (H, 10)
    c2 = ((q @ kb.astype(np.float64)) / np.sqrt(HEAD)).astype(np.float32)  # (10,)
    # device layout: (128, HC*NPR), free index = c*NPR + n for h = c*128 + p
    import ml_dtypes

    w2st = np.ascontiguousarray(
        w2s.reshape(HC, 128, NPR).transpose(1, 0, 2).reshape(128, HC * NPR)
    ).astype(ml_dtypes.bfloat16)
    return w2st, c2.reshape(NPR, 1)


def _prepare_in_maps(
    relevance, hidden_states_src, prompts, label_prompts,
    ref_qw, ref_qb, ref_kw, ref_kb, **_unused,
):
    import ml_dtypes

    bf16 = ml_dtypes.bfloat16
    relevance = np.asarray(relevance, dtype=np.float32)
    hidden_states_src = np.asarray(hidden_states_src, dtype=np.float32)
    prompts = np.asarray(prompts, dtype=np.float32)
    label_prompts = np.asarray(label_prompts, dtype=np.float32)

    w2st, c2 = _host_fold(
        prompts, label_prompts,
        np.asarray(ref_qw, np.float32), np.asarray(ref_qb, np.float32),
        np.asarray(ref_kw, np.float32), np.asarray(ref_kb, np.float32),
    )
    prom_bf = np.ascontiguousarray(prompts).astype(bf16)
    hid_bf = hidden_states_src.astype(bf16)
    # rel rows = [1-r, r] @ label_prompts, exactly the tiny hidden_rel
    rel2 = np.stack([1.0 - relevance, relevance], axis=1).astype(np.float64)
    rel_rows = (rel2 @ label_prompts.astype(np.float64)).astype(np.float32)

    in_maps = []
    for core in range(NCORES):
        sl = slice(core * BLOC, (core + 1) * BLOC)
        in_maps.append(
            {
                "hid": np.ascontiguousarray(hid_bf[sl]),
                "relr": rel_rows[sl].astype(bf16).reshape(1, BLOC * H),
                "prom": prom_bf,
                "c2f": c2,
                "w2st": w2st,
            }
        )
    return in_maps


def _get_module():
    if "nc" not in _CACHE:
        _CACHE["nc"] = _build_module()
    return _CACHE["nc"]


def kernel(**inputs):
    from concourse.bass_utils import run_bass_kernel_spmd

    nc = _get_module()
    in_maps = _prepare_in_maps(**inputs)
    res = run_bass_kernel_spmd(nc, in_maps, list(range(NCORES)))
    return np.concatenate(
        [res.results[c]["out"] for c in range(NCORES)], axis=0
    ).astype(np.float32)


# revision 23
# speedup vs baseline: 1.1258x; 1.0558x over previous
"""Bass/Trainium2 kernel for nn_DocRelPrompt.

Reference computation (B=64, L=512, H=768, HEAD=64, N_PROMPTS=10, N_LBL=2):
    rel2 = stack([1-r, r], 1)                   # (B, 2)
    hidden_rel = rel2 @ label_prompts           # (B, H)
    Q  = prompts @ ref_qw.T + ref_qb            # (10, HEAD)
    K  = hid @ ref_kw.T + ref_kb                # (B, L, HEAD)
    scores[b,n] = mean_l(Q[n] . K[b,l]) / 8
                = (hsum[b] . (Q@ref_kw)[n] / (512*8)) + (Q[n].ref_kb)/8
    gate = sigmoid(scores)                      # (B, 10)
    doc  = prompts[None] * gate[..., None]      # (B, 10, H)
    out  = concat([doc, hid + hidden_rel[:,None,:]], axis=1)   # (B, 522, H)

(The `_rel_prompts` branch of the reference is computed but unused, so it is
skipped entirely.)

Sharding: pure data-parallel over batch, 8 cores x 8 batches.  The tiny
prompt/weight tensors are folded on the host into W2s (768,10, bf16) and
c2 (10,) and replicated.

The kernel is memory-bound and the correctness gate is rel_err < 2e-2, so
the bulk streams run in bf16 end to end: the host downcasts hid to bf16
(halving the in-stream), the device computes body = hid + rel and the doc
gate entirely from the bf16 tiles, writes a bf16 out tensor (halving the
out-stream), and the host upcasts to f32.  Simulated numerics: rel err
~5.6e-3 vs the 2e-2 gate.

DRAM layout uses l = 4p + t ("(p t)") so every DMA touches per-partition
contiguous runs (6 KB full-tile / 3 KB half-tile) instead of 1.5 KB rows.

Device work per core, per batch (DMA-bound; ~13 MB HBM traffic total):
  - two half-tile hid loads (128, 2, 768) bf16 on the SP HWDGE ring, which
    carries nothing else (consts ride the ACT ring);
  - PE: hsum[b] (1, 768) via ones-stationary matmuls PSUM-accumulated over
    t-slices, ACT downcast, then 6 PE transposes build hsumT columns;
  - per-batch-PAIR gate tail: 6 bf16 matmuls accumulate scores, ACT
    sigmoid(+c2), DVE tensor_scalar doc rows (bf16), doc DMA;
  - DVE: rel row = db*r_b + lp0b (scalar_tensor_tensor, bf16), then per
    half-tile an in-place bf16 tensor_tensor body = hid + rel and a 0.4 MB
    body DMA on the ACT HWDGE ring.
"""

import numpy as np

B, L, H, HEAD, NPR, NLBL = 64, 512, 768, 64, 10, 2
NCORES = 8
BLOC = B // NCORES          # 8 batches per core
LT = L // 128               # 4 t-slices (l = 4p + t)
HC = H // 128               # 6 H-chunks of 128

_CACHE = {}


def _build_module():
    from contextlib import ExitStack

    import concourse.bacc as bacc
    import concourse.mybir as mybir
    from concourse.tile import TileContext

    dt = mybir.dt.float32
    bf = mybir.dt.bfloat16
    ADD = mybir.AluOpType.add

    # Bacc (not raw Bass): its compile() legalizes sync waits — TRN2
    # instructions carry at most one wait, extras become event-sem waits.
    nc = bacc.Bacc("TRN2", target_bir_lowering=False, debug=False)
    hid = nc.dram_tensor("hid", [BLOC, L, H], bf, kind="ExternalInput")
    # constants: relr (the 8 per-batch rel rows, host-computed, 12 KB —
    # partition-broadcast on-chip per batch via PE ones-matmuls instead of
    # streaming a 395 KB pre-broadcast tensor from HBM), prom (prompts,
    # bf16), c2f (sigmoid bias, f32), w2st (folded score weights, bf16) —
    # all on the ACT ring so the SP ring is a pure hid stream
    relr = nc.dram_tensor("relr", [1, BLOC * H], bf, kind="ExternalInput")
    prom = nc.dram_tensor("prom", [NPR, H], bf, kind="ExternalInput")
    c2f = nc.dram_tensor("c2f", [NPR, 1], dt, kind="ExternalInput")
    w2st = nc.dram_tensor("w2st", [128, HC * NPR], bf, kind="ExternalInput")
    out = nc.dram_tensor("out", [BLOC, NPR + L, H], bf, kind="ExternalOutput")

    # l = 4p + t: per-partition contiguous DRAM runs (4*H elems full tile)
    hid_r = hid[:].rearrange("b (p t) h -> b p t h", t=LT)
    body_r = out[:, NPR:, :].rearrange("b (p t) h -> b p t h", t=LT)

    with TileContext(nc) as tc, ExitStack() as ctx:
        const = ctx.enter_context(tc.tile_pool(name="const", bufs=1))
        hidp = ctx.enter_context(tc.tile_pool(name="hidp", bufs=8))
        bodyp = ctx.enter_context(tc.tile_pool(name="bodyp", bufs=14))
        relp = ctx.enter_context(tc.tile_pool(name="relp", bufs=4))
        hstp = ctx.enter_context(tc.tile_pool(name="hstp", bufs=2, space="PSUM"))
        hsbp = ctx.enter_context(tc.tile_pool(name="hsbp", bufs=2))
        rbcp = ctx.enter_context(tc.tile_pool(name="rbcp", bufs=2, space="PSUM"))
        scop = ctx.enter_context(tc.tile_pool(name="scop", bufs=1, space="PSUM"))
        warmp = ctx.enter_context(tc.tile_pool(name="warmp", bufs=1, space="PSUM"))
        small = ctx.enter_context(tc.tile_pool(name="small", bufs=1))

        ones_bf = const.tile([128, 1], bf)
        nc.vector.memset(ones_bf[:], 1.0)
        ones_row = const.tile([1, 128], bf)
        nc.vector.memset(ones_row[:], 1.0)

        # the SP HWDGE ring is FIFO and carries only the hid stream; issue
        # the first loads immediately
        t_ins = []
        for b in range(2):
            t_in = hidp.tile([128, LT, H], bf, tag="hid")
            nc.sync.dma_start(t_in[:, 0:2], hid_r[b][:, 0:2])
            nc.sync.dma_start(t_in[:, 2:4], hid_r[b][:, 2:4])
            t_ins.append(t_in)

        # consts on the ACT ring (empty this early; first rel use is after
        # the first full hid load anyway)
        relr_sb = const.tile([1, BLOC * H], bf)
        nc.scalar.dma_start(relr_sb[:], relr[:])
        w2st_sb = const.tile([128, HC * NPR], bf)
        nc.scalar.dma_start(w2st_sb[:], w2st[:])
        prom_sb = const.tile([NPR, H], bf)
        nc.scalar.dma_start(prom_sb[:], prom[:])
        c2_sb = const.tile([NPR, 1], dt)
        nc.scalar.dma_start(c2_sb[:], c2f[:])

        # Warm-up matmuls: sync the PE against the DVE memsets and the w2st
        # DMA one dependency at a time — matmuls tolerate few sync waits.
        scrap_ps = warmp.tile([128, 1], dt)
        nc.tensor.matmul(scrap_ps[0:1, :], ones_bf[:], ones_bf[:],
                         start=True, stop=True)
        nc.tensor.matmul(scrap_ps[0:NPR, :], w2st_sb[:, 0:NPR], ones_bf[:],
                         start=True, stop=True)
        # preload the sigmoid table during boot (1.3us ACT_TABLE_LOAD that
        # would otherwise land on the critical tail)
        sig_warm = small.tile([1, 1], dt)
        nc.scalar.activation(sig_warm[:], ones_bf[0:1, 0:1],
                             func=mybir.ActivationFunctionType.Sigmoid)

        # all doc rows accumulate here; ONE doc DMA at the end of the kernel
        # (4 interleaved doc DMAs would each cost a serialized ~0.6us
        # dispatch slot in the congested out-queue)
        doc_all = const.tile([NPR, BLOC, H], bf)

        # rel[b] rows (host-computed) partition-broadcast via PE
        # ones-matmuls (f32 PSUM, split 512/256 on the bank edge) + one ACT
        # downcast each.  Scheduled TWO batches ahead of use: emitted
        # in-loop they queue behind a full batch of hsum chunks on the
        # in-order PE/ACT queues, which made the last batch's adds start
        # ~4us after its data had landed.
        rel_ts = [None] * BLOC

        def rel_bcast(b):
            rel_ps = rbcp.tile([128, H], dt, tag="relps")
            nc.tensor.matmul(rel_ps[:, 0:512], ones_row[:],
                             relr_sb[0:1, b * H : b * H + 512],
                             start=True, stop=True)
            nc.tensor.matmul(rel_ps[:, 512:H], ones_row[:],
                             relr_sb[0:1, b * H + 512 : (b + 1) * H],
                             start=True, stop=True)
            rel_t = relp.tile([128, H], bf, tag="relsb")
            nc.scalar.copy(rel_t[:], rel_ps[:])
            rel_ts[b] = rel_t

        rel_bcast(0)
        rel_bcast(1)

        for b in range(BLOC):
            if b < 2:
                t_in = t_ins[b]
            else:
                t_in = hidp.tile([128, LT, H], bf, tag="hid")
                # half-tile loads: downstream compute starts earlier and
                # the outbound stream interleaves more smoothly
                nc.sync.dma_start(t_in[:, 0:2], hid_r[b][:, 0:2])
                nc.sync.dma_start(t_in[:, 2:4], hid_r[b][:, 2:4])

            rel_t = rel_ts[b]

            # stage 1a: hsumT (128, HC) — hid-chunk-stationary matmuls with
            # a ones moving vector put the h-chunk sums directly ON the
            # partitions (PSUM f32 accumulation over the 4 t-slices).  This
            # skips the (1,768) ACT downcast + 6 PE transposes of the
            # classic hsum formulation — the ACT queue is the congested one
            # and those copies sat in the last batch's critical chain.
            hsT_ps = hstp.tile([128, HC], dt, tag="hsT")
            for c in range(HC):
                for t in range(LT):
                    nc.tensor.matmul(
                        hsT_ps[:, c : c + 1],
                        t_in[:, t, c * 128 : (c + 1) * 128],
                        ones_bf[:],
                        start=(t == 0), stop=(t == LT - 1),
                    )

            # body = hid + rel (separate output tile, NOT in place: an
            # in-place add has a WAR hazard against the PE hsum reads of the
            # same buffer, which would chain PE latency into the body path),
            # free-dim broadcast of rel over t-slices.  Mid-stream outs ride
            # the ACT HWDGE ring so they don't queue behind in-loads on the
            # SP ring; the tail splits across BOTH rings (the SP ring is
            # idle once the in-stream ends) and the last half goes in
            # quarters so the final transfer chases a half-size add.
            chunks = [(slice(0, 2), nc.scalar), (slice(2, 4), nc.scalar)]
            if b == 7:
                chunks = [(slice(0, 2), nc.scalar), (slice(2, 3), nc.scalar),
                          (slice(3, 4), nc.sync)]
            elif b == 6:
                chunks = [(slice(0, 2), nc.sync), (slice(2, 4), nc.scalar)]
            for sl, eng in chunks:
                n = sl.stop - sl.start
                t_body = bodyp.tile([128, n, H], bf, tag=f"body{n}")
                nc.vector.tensor_tensor(
                    t_body[:], t_in[:, sl],
                    rel_t[:, None, :].broadcast_to([128, n, H]),
                    ADD,
                )
                eng.dma_start(body_r[b][:, sl], t_body[:])

            # broadcast the rel row needed two iterations from now (after
            # the body block, so its ACT copy sits behind this batch's out
            # dispatches in the Scalar queue, not ahead of them)
            if b + 2 < BLOC:
                rel_bcast(b + 2)

            # stage 1b: one tiny ACT downcast (6 elems/partition) per batch,
            # emitted AFTER the body block so it never head-of-line blocks
            # an out-DMA dispatch behind it in the Scalar queue.
            if b % 2 == 0:
                hsT_p = hsbp.tile([128, HC, 2], bf, tag="hstp")
            nc.scalar.copy(hsT_p[:, :, b % 2], hsT_ps[:])

            # gate pipeline per batch PAIR — score columns are independent;
            # pairing halves the tiny stage-2 matmuls / sigmoids.
            if b % 2 == 1:
                score_p = scop.tile([NPR, 2], dt, tag="scorep")
                for c in range(HC):
                    nc.tensor.matmul(
                        score_p[:], w2st_sb[:, c * NPR : (c + 1) * NPR],
                        hsT_p[:, c, 0:2],
                        start=(c == 0), stop=(c == HC - 1),
                    )
                gate_p = hsbp.tile([NPR, 2], dt, tag="gatep")
                nc.scalar.activation(
                    gate_p[:], score_p[:],
                    func=mybir.ActivationFunctionType.Sigmoid,
                    bias=c2_sb, scale=1.0,
                )
                for j in range(2):
                    nc.vector.tensor_scalar(
                        doc_all[:, b - 1 + j, :], prom_sb,
                        gate_p[:, j : j + 1], None,
                        mybir.AluOpType.mult,
                    )

        # single doc DMA for all 8 batches, on the SP ring (idle by now —
        # the in-stream is done well before the last gate chain resolves)
        nc.sync.dma_start(
            out[:, 0:NPR, :].transpose([1, 0, 2]), doc_all[:]
        )

    nc.compile()
    return nc


def _host_fold(prompts, label_prompts, qw, qb, kw, kb):
    """Fold the tiny projection weights on the host.

    scores[b, n] = hsum[b] . W2s[:, n] + c2[n], with W2s/c2 absorbing the
    1/L mean pooling and the 1/sqrt(HEAD) scaling.
    """
    q = prompts.astype(np.float64) @ qw.astype(np.float64).T + qb.astype(np.float64)
    w2 = q @ kw.astype(np.float64)                               # (10, H)
    w2s = (w2.T / (L * np.sqrt(HEAD))).astype(np.float32)        # (H, 10)
    c2 = ((q @ kb.astype(np.float64)) / np.sqrt(HEAD)).astype(np.float32)  # (10,)
    # device layout: (128, HC*NPR), free index = c*NPR + n for h = c*128 + p
    import ml_dtypes

    w2st = np.ascontiguousarray(
        w2s.reshape(HC, 128, NPR).transpose(1, 0, 2).reshape(128, HC * NPR)
    ).astype(ml_dtypes.bfloat16)
    return w2st, c2.reshape(NPR, 1)


def _prepare_in_maps(
    relevance, hidden_states_src, prompts, label_prompts,
    ref_qw, ref_qb, ref_kw, ref_kb, **_unused,
):
    import ml_dtypes

    bf16 = ml_dtypes.bfloat16
    relevance = np.asarray(relevance, dtype=np.float32)
    hidden_states_src = np.asarray(hidden_states_src, dtype=np.float32)
    prompts = np.asarray(prompts, dtype=np.float32)
    label_prompts = np.asarray(label_prompts, dtype=np.float32)

    w2st, c2 = _host_fold(
        prompts, label_prompts,
        np.asarray(ref_qw, np.float32), np.asarray(ref_qb, np.float32),
        np.asarray(ref_kw, np.float32), np.asarray(ref_kb, np.float32),
    )
    prom_bf = np.ascontiguousarray(prompts).astype(bf16)
    hid_bf = hidden_states_src.astype(bf16)
    # rel rows = [1-r, r] @ label_prompts, exactly the tiny hidden_rel
    rel2 = np.stack([1.0 - relevance, relevance], axis=1).astype(np.float64)
    rel_rows = (rel2 @ label_prompts.astype(np.float64)).astype(np.float32)

    in_maps = []
    for core in range(NCORES):
        sl = slice(core * BLOC, (core + 1) * BLOC)
        in_maps.append(
            {
                "hid": np.ascontiguousarray(hid_bf[sl]),
                "relr": rel_rows[sl].astype(bf16).reshape(1, BLOC * H),
                "prom": prom_bf,
                "c2f": c2,
                "w2st": w2st,
            }
        )
    return in_maps


def _get_module():
    if "nc" not in _CACHE:
        _CACHE["nc"] = _build_module()
    return _CACHE["nc"]


def kernel(**inputs):
    from concourse.bass_utils import run_bass_kernel_spmd

    nc = _get_module()
    in_maps = _prepare_in_maps(**inputs)
    res = run_bass_kernel_spmd(nc, in_maps, list(range(NCORES)))
    return np.concatenate(
        [res.results[c]["out"] for c in range(NCORES)], axis=0
    ).astype(np.float32)
